# revision 14
# baseline (speedup 1.0000x reference)
"""Trainium2 Bass kernel for nn_MultiHeadLinearAttention.

Full-input contract: kernel(**inputs) takes the unsharded numpy inputs and
returns the full output. Internally: data-parallel over batch across the 8
NeuronCores (B == 8, one batch element per core), no collectives.

Per-core math (S=2048, E=2048, H=16, d=128), bf16 matmuls + fp32 PSUM:
  Pass A (per head):
    qT  = Wq[h]-stationary route            -> pqT [d, S] (kept for all heads)
    k   = xT-chunk-stationary route         -> pk  [S, d] (natural, transient)
    phi(x) = elu(x)+1 = min(exp(x), 1 + relu(x))   (exact identity)
    Gram trick: G[d', d] = sum_s xh[s,d'] pk[s,d]  (16 accum. matmuls)
                kv = G^T @ Wv[h]  (one matmul; never materializes v)
    ksum[d]  = sum_s pk[s,d]  (N=1 matmuls into kv psum col 128)
  Pass B (per s-chunk sc):
    for each head: nd[s,129] = pqT-chunk^T @ [kv|ksum]; ctx_h = num * inv
    one batched xbar-DMA transpose of ctx_sc[s, (h d)] -> ctxT_sc[d, (h s)]
    out[sc, :] = sum_h ctxT_sc[:, h, :]^T @ Wo[h-rows, :]  (Wo streamed in
    eo-quarters)
Host does: x transpose + bf16 casts + weight packing + bias add + gather.
"""

import numpy as np
import ml_dtypes

import concourse.bass as bass
import concourse.mybir as mybir
import concourse.tile as tile
from concourse import bacc
from concourse.bass_utils import run_bass_kernel_spmd

S = 2048
E = 2048
H = 16
D = 128
N_CORES = 8
NCH = S // 128  # 16 s-chunks

F32 = mybir.dt.float32
BF16 = mybir.dt.bfloat16
AF = mybir.ActivationFunctionType
ALU = mybir.AluOpType

_CACHED = {}


def _phi(nc, work, psum_tile, dst):
    """phi(x) = min(exp(x), 1 + relu(x)) from a [128,1024] PSUM tile into
    bf16 SBUF dst (exact identity for elu(x)+1)."""
    e = work.tile([128, 1024], BF16, tag="e", bufs=2)
    nc.scalar.activation(e[:], psum_tile[:], AF.Exp)
    t = work.tile([128, 1024], BF16, tag="t", bufs=2)
    nc.vector.tensor_scalar(t[:], psum_tile[:], 0.0, 1.0, ALU.max, ALU.add)
    nc.vector.tensor_tensor(dst, e[:], t[:], ALU.min)


def build_module():
    nc = bacc.Bacc("TRN2", target_bir_lowering=False, debug=False,
                   num_devices=N_CORES)

    xT = nc.dram_tensor("xT", [E, S], BF16, kind="ExternalInput")
    xn = nc.dram_tensor("xn", [H, 128, NCH * 128], BF16,
                        kind="ExternalInput")
    wq = nc.dram_tensor("wq", [D, H * D], BF16, kind="ExternalInput")
    wkv = nc.dram_tensor("wkv", [D, H * 2 * D], BF16, kind="ExternalInput")
    wo = nc.dram_tensor("wo", [D, H * E], BF16, kind="ExternalInput")
    out = nc.dram_tensor("out", [S, E], F32, kind="ExternalOutput")

    with tile.TileContext(nc) as tc:
        with (
            tc.tile_pool(name="const", bufs=1) as const,
            tc.tile_pool(name="work", bufs=2) as work,
            tc.tile_pool(name="psum", bufs=2, space="PSUM") as psum,
        ):
            wq_sb = const.tile([128, H * D], BF16)
            # split so the first q-proj matmul (needs cols 0:128 only) can
            # start as soon as the small slice lands
            nc.sync.dma_start(out=wq_sb[:, 0:128], in_=wq[:, 0:128])
            wkv_sb = const.tile([128, H * 2 * D], BF16)
            ones_col = const.tile([128, 1], BF16)
            nc.vector.memset(ones_col[:], 1.0)
            warm = const.tile([128, 1], F32)
            nc.vector.memset(warm[:], 0.0)
            nc.scalar.activation(warm[:], warm[:], AF.Exp)
            pqT = const.tile([128, H * S], BF16)      # all heads
            kv_all = const.tile([128, H * 129], BF16)  # all heads [kv|ksum]

            ctxT_tiles = {}
            ctx_tiles = {}

            def emit_nd_chunk(sc, h):
                ctx_sc = ctx_tiles[sc]
                # pass-B chunks alternate between the two 1-bank rings for
                # effective depth 4; pass-A chunks stay off "q2" (qp owns it)
                ndp = psum.tile([128, 129], F32,
                                tag="g" if (sc < 2 or h % 2 == 0) else "q2",
                                bufs=2)
                nc.tensor.matmul(
                    ndp[:],
                    pqT[:, h * S + sc * 128:h * S + (sc + 1) * 128],
                    kv_all[:, h * 129:(h + 1) * 129],
                    start=True, stop=True)
                inv = work.tile([128, 1], F32, tag="inv", bufs=4)
                nc.vector.reciprocal(inv[:], ndp[:, 128:129])
                dst = ctx_sc[:, h, :]
                if (sc + h) % 2 == 0:
                    nc.scalar.activation(dst, ndp[:, 0:128], AF.Copy,
                                         scale=inv[:, 0:1])
                else:
                    nc.vector.tensor_scalar(dst, ndp[:, 0:128],
                                            inv[:, 0:1], None, ALU.mult)

            xh2_tiles = {}

            xh2_pending = []

            def prefetch_q2(h, quarter):
                xh2 = work.tile([128, 512], BF16, tag="xh2", bufs=3)
                nc.sync.dma_start(
                    out=xh2[:],
                    in_=xT[h * 128:(h + 1) * 128,
                           quarter * 512:(quarter + 1) * 512])
                xh2_pending.append(xh2)

            def emit_q2_half(h, quarter):
                xh2 = xh2_pending.pop(0)
                q2p = psum.tile([128, 512], F32, tag="q2", bufs=2)
                nc.tensor.matmul(
                    q2p[:], wq_sb[:, h * 128:(h + 1) * 128],
                    xh2[:], start=True, stop=True)
                base = h * S + quarter * 512
                # Both PSUM reads on Act (releases q2p fast); DVE ops are
                # bf16-SBUF (4x/2x eligible) and off the q2p critical path.
                e2 = work.tile([128, 512], BF16, tag="e", bufs=2)
                nc.scalar.activation(e2[:], q2p[:], AF.Exp)
                r2 = work.tile([128, 512], BF16, tag="t", bufs=2)
                nc.scalar.activation(r2[:], q2p[:], AF.Relu)
                nc.vector.scalar_tensor_tensor(
                    pqT[:, base:base + 512], r2[:], 1.0, e2[:],
                    ALU.add, ALU.min)

            def emit_transpose(sc):
                ctx_sc = ctx_tiles.pop(sc)
                ctxT_sc = work.tile([128, H, 128], BF16, tag="ctxT", bufs=2)
                nc.sync.dma_start(
                    out=ctxT_sc[:],
                    in_=ctx_sc[:].rearrange("p h j -> p (h j)"),
                    transpose=True)
                ctxT_tiles[sc] = ctxT_sc

            def emit_nd(sc):
                ctx_sc = work.tile([128, H, 128], BF16, tag="ctx", bufs=2)
                ctx_tiles[sc] = ctx_sc
                for h in range(H):
                    emit_nd_chunk(sc, h)
                emit_transpose(sc)

            def emit_accum_eop(ctxT_sc, sc, eop, mid_hook=None, nd_hook=None):
                for eop in [eop]:
                    pa = psum.tile([128, 1024], F32, tag="pj", bufs=2)
                    for h in range(H):
                        if h in (5, 11) and mid_hook is not None:
                            mid_hook()
                        if nd_hook is not None:
                            nd_hook(h)
                        for half in range(2):
                            eoq = eop * 2 + half
                            nc.tensor.matmul(
                                pa[:, half * 512:(half + 1) * 512],
                                ctxT_sc[:, h, :], woq_tiles[eoq][:, h, :],
                                start=(h == 0), stop=(h == H - 1))
                    out_t = work.tile([128, 1024], F32, tag="outsb", bufs=2)
                    if eop % 2 == 0:
                        nc.scalar.activation(out_t[:], pa[:], AF.Copy)
                    else:
                        nc.vector.tensor_copy(out_t[:], pa[:])
                    nc.sync.dma_start(
                        out=out[sc * 128:(sc + 1) * 128,
                                eop * 1024:(eop + 1) * 1024],
                        in_=out_t[:])

            wo_v = wo[:].rearrange("p (h q x) -> p h q x", q=4, x=512)
            woq_tiles = []

            # -------- Pass A (software-pipelined over heads) --------------
            # stage 1 (head h): q/k projections + phi; stage 2 (head h-1):
            # Gram/ksum/kv + early nd chunks.  The one-head lag hides the
            # phi latency so the PE never waits on the DVE/Act/Pool chains.
            pa_state = {}

            def emit_qk(h):
                xhT = work.tile([128, S], BF16, tag="xhT", bufs=2)
                if h == 0:
                    nc.sync.dma_start(out=xhT[:, 0:512], in_=xT[0:128, 0:512])
                    nc.sync.dma_start(out=wkv_sb[:, 0:256], in_=wkv[:, 0:256])
                    nc.sync.dma_start(out=xhT[:, 512:S],
                                      in_=xT[0:128, 512:S])
                else:
                    nc.sync.dma_start(out=xhT[:],
                                      in_=xT[h * 128:(h + 1) * 128, :])
                xn_sb = work.tile([128, NCH, 128], BF16, tag="xn", bufs=2)
                nc.sync.dma_start(
                    out=xn_sb[:],
                    in_=xn[h].rearrange("p (c j) -> p c j", j=128))
                if h == 0:
                    nc.sync.dma_start(out=wq_sb[:, 128:H * D],
                                      in_=wq[:, 128:H * D])
                    nc.sync.dma_start(out=wkv_sb[:, 256:],
                                      in_=wkv[:, 256:])

                # q quarter 0 + phi -> pqT[h, 0:512]; quarters 1-3 deferred
                # into pass B (emit_q2_half)
                qp = psum.tile([128, 512], F32, tag="q2", bufs=2)
                nc.tensor.matmul(
                    qp[:], wq_sb[:, h * 128:(h + 1) * 128],
                    xhT[:, 0:512], start=True, stop=True)
                eq = work.tile([128, 512], BF16, tag="e", bufs=2)
                nc.scalar.activation(eq[:], qp[:], AF.Exp)
                tq = work.tile([128, 512], BF16, tag="t", bufs=2)
                nc.vector.tensor_scalar(tq[:], qp[:], 0.0, 1.0,
                                        ALU.max, ALU.add)
                nc.vector.tensor_tensor(pqT[:, h * S:h * S + 512],
                                        eq[:], tq[:], ALU.min)

                # k (natural layout) + phi -> pk; mins go to the (otherwise
                # idle) GPSIMD engine to unload the DVE
                pk = work.tile([128, S], BF16, tag="pk", bufs=2)
                for j in range(2):
                    kp = psum.tile([128, 1024], F32, tag="pj", bufs=2)
                    for c in range(8):
                        sc = j * 8 + c
                        nc.tensor.matmul(
                            kp[:, c * 128:(c + 1) * 128],
                            xhT[:, sc * 128:(sc + 1) * 128],
                            wkv_sb[:, h * 256:h * 256 + 128],
                            start=True, stop=True)
                    e = work.tile([128, 1024], BF16, tag="e", bufs=2)
                    nc.scalar.activation(e[:], kp[:], AF.Exp)
                    t = work.tile([128, 1024], BF16, tag="t", bufs=2)
                    nc.vector.tensor_scalar(t[:], kp[:], 0.0, 1.0,
                                            ALU.max, ALU.add)
                    nc.gpsimd.tensor_tensor(
                        pk[:, j * 1024:(j + 1) * 1024], e[:], t[:], ALU.min)
                pa_state[h] = (pk, xn_sb)

            def emit_gram(h):
                pk, xn_sb = pa_state.pop(h)
                # one 1-bank psum packs G | kv | ksum: [0:128]=G, [128:256]=kv,
                # [256:257]=ksum, so kv_all[h] copies out in one op
                gkv = psum.tile([128, 257], F32, tag="g", bufs=2)
                for c in range(NCH):
                    nc.tensor.matmul(gkv[:, 0:128], xn_sb[:, c, :],
                                     pk[:, c * 128:(c + 1) * 128],
                                     start=(c == 0), stop=(c == NCH - 1))
                    nc.tensor.matmul(gkv[:, 256:257],
                                     pk[:, c * 128:(c + 1) * 128],
                                     ones_col[:],
                                     start=(c == 0), stop=(c == NCH - 1))
                g_sb = work.tile([128, 128], BF16, tag="gsb")
                nc.scalar.activation(g_sb[:], gkv[:, 0:128], AF.Copy)
                nc.tensor.matmul(gkv[:, 128:256], g_sb[:],
                                 wkv_sb[:, h * 256 + 128:h * 256 + 256],
                                 start=True, stop=True)
                nc.scalar.activation(
                    kv_all[:, h * 129:(h + 1) * 129], gkv[:, 128:257],
                    AF.Copy)
                if h == 0:
                    for esc in range(2):
                        ctxe = work.tile([128, H, 128], BF16,
                                         tag="ctx", bufs=2)
                        ctx_tiles[esc] = ctxe
                for esc in range(2):
                    emit_nd_chunk(esc, h)

            for h in range(H):
                emit_qk(h)
                if h > 0:
                    emit_gram(h - 1)
                if h == 10:
                    for eoq in range(4):
                        woq = work.tile([128, H, 512], BF16,
                                        tag=f"woq{eoq}", bufs=1)
                        nc.sync.dma_start(out=woq[:], in_=wo_v[:, :, eoq, :])
                        woq_tiles.append(woq)
            emit_gram(H - 1)

            emit_transpose(0)
            emit_transpose(1)
            q2q = [(h, q) for q in (1, 2, 3) for h in range(H)]
            q2q.reverse()  # pop() from the front

            q2pf = list(q2q)

            def drain_q2(n):
                for _ in range(n):
                    # keep two DMAs in flight ahead of the compute pieces
                    for _k in range(2 - len(xh2_pending) + 0):
                        pass
                    while q2pf and len(xh2_pending) < 2:
                        hq = q2pf.pop()
                        prefetch_q2(*hq)
                    if q2q:
                        h, half = q2q.pop()
                        emit_q2_half(h, half)

            for sc in range(2, NCH + 2):
                drain_q2(1)
                # nd chunks for sc interleave into the Wo stream of sc-2 so
                # the PE never head-of-line blocks on the nd scale chain
                if sc < NCH:
                    ctx_sc = work.tile([128, H, 128], BF16, tag="ctx", bufs=2)
                    ctx_tiles[sc] = ctx_sc
                    nd_hook = lambda h, _sc=sc: emit_nd_chunk(_sc, h)
                else:
                    nd_hook = None
                ctxT_sc = ctxT_tiles.pop(sc - 2)
                emit_accum_eop(ctxT_sc, sc - 2, 0,
                               mid_hook=lambda: drain_q2(1),
                               nd_hook=nd_hook)
                if sc < NCH:
                    emit_transpose(sc)
                drain_q2(1)
                emit_accum_eop(ctxT_sc, sc - 2, 1,
                               mid_hook=lambda: drain_q2(1))
                drain_q2(2)

    nc.compile()
    return nc


def get_module():
    if "nc" not in _CACHED:
        _CACHED["nc"] = build_module()
    return _CACHED["nc"]


def _bf16(a):
    return np.ascontiguousarray(a).astype(ml_dtypes.bfloat16)


def prepare_in_maps(inputs, Wq, Wk, Wv, Wo, bo):
    """Host-side shard + layout prep. Returns per-core input maps."""
    wq_p = _bf16(np.transpose(np.asarray(Wq), (1, 0, 2)).reshape(D, H * D))
    wkv = np.concatenate([np.asarray(Wk), np.asarray(Wv)], axis=2)  # (H,d,2d)
    wkv_p = _bf16(np.transpose(wkv, (1, 0, 2)).reshape(D, H * 2 * D))
    wo_p = _bf16(np.transpose(np.asarray(Wo).reshape(H, D, E),
                              (1, 0, 2)).reshape(D, H * E))
    in_maps = []
    for b in range(N_CORES):
        xb = np.asarray(inputs[b])
        # xn packed per head: xn[h][p, c*128+j] = x[c*128+p, h*128+j]
        xn_p = _bf16(np.transpose(xb.reshape(NCH, 128, H, D),
                                  (2, 1, 0, 3)).reshape(H, 128, NCH * D))
        in_maps.append({"xT": _bf16(xb.T), "xn": xn_p,
                        "wq": wq_p, "wkv": wkv_p, "wo": wo_p})
    return in_maps


def kernel(inputs, Wq, Wk, Wv, Wo, bo):
    B = inputs.shape[0]
    assert B == N_CORES and inputs.shape[1:] == (S, E)
    nc = get_module()
    in_maps = prepare_in_maps(inputs, Wq, Wk, Wv, Wo, bo)
    res = run_bass_kernel_spmd(nc, in_maps, list(range(N_CORES)))
    outs = np.stack([res.results[b]["out"] for b in range(N_CORES)], axis=0)
    return (outs + np.asarray(bo, dtype=np.float32)[None, None, :]).astype(
        np.float32)



# revision 17
# speedup vs baseline: 1.0425x; 1.0425x over previous
"""Trainium2 Bass kernel for nn_MultiHeadLinearAttention.

Full-input contract: kernel(**inputs) takes the unsharded numpy inputs and
returns the full output. Internally: data-parallel over batch across the 8
NeuronCores (B == 8, one batch element per core), no collectives.

Per-core math (S=2048, E=2048, H=16, d=128), bf16 matmuls + fp32 PSUM:
  Pass A (per head):
    qT  = Wq[h]-stationary route            -> pqT [d, S] (kept for all heads)
    k   = xT-chunk-stationary route         -> pk  [S, d] (natural, transient)
    phi(x) = elu(x)+1 = min(exp(x), 1 + relu(x))   (exact identity)
    Gram trick: G[d', d] = sum_s xh[s,d'] pk[s,d]  (16 accum. matmuls)
                kv = G^T @ Wv[h]  (one matmul; never materializes v)
    ksum[d]  = sum_s pk[s,d]  (N=1 matmuls into kv psum col 128)
  Pass B (per s-chunk sc):
    for each head: nd[s,129] = pqT-chunk^T @ [kv|ksum]; ctx_h = num * inv
    one batched xbar-DMA transpose of ctx_sc[s, (h d)] -> ctxT_sc[d, (h s)]
    out[sc, :] = sum_h ctxT_sc[:, h, :]^T @ Wo[h-rows, :]  (Wo streamed in
    eo-quarters)
Host does: x transpose + bf16 casts + weight packing + bias add + gather.
"""

import numpy as np
import ml_dtypes

import concourse.bass as bass
import concourse.mybir as mybir
import concourse.tile as tile
from concourse import bacc
from concourse.bass_utils import run_bass_kernel_spmd

S = 2048
E = 2048
H = 16
D = 128
N_CORES = 8
NCH = S // 128  # 16 s-chunks

F32 = mybir.dt.float32
BF16 = mybir.dt.bfloat16
AF = mybir.ActivationFunctionType
ALU = mybir.AluOpType

_CACHED = {}


def _phi(nc, work, psum_tile, dst):
    """phi(x) = min(exp(x), 1 + relu(x)) from a [128,1024] PSUM tile into
    bf16 SBUF dst (exact identity for elu(x)+1)."""
    e = work.tile([128, 1024], BF16, tag="e", bufs=2)
    nc.scalar.activation(e[:], psum_tile[:], AF.Exp)
    t = work.tile([128, 1024], BF16, tag="t", bufs=2)
    nc.vector.tensor_scalar(t[:], psum_tile[:], 0.0, 1.0, ALU.max, ALU.add)
    nc.vector.tensor_tensor(dst, e[:], t[:], ALU.min)


def build_module():
    nc = bacc.Bacc("TRN2", target_bir_lowering=False, debug=False,
                   num_devices=N_CORES)

    xT = nc.dram_tensor("xT", [E, S], BF16, kind="ExternalInput")
    xn = nc.dram_tensor("xn", [H, 128, NCH * 128], BF16,
                        kind="ExternalInput")
    wq = nc.dram_tensor("wq", [D, H * D], BF16, kind="ExternalInput")
    wkv = nc.dram_tensor("wkv", [D, H * 2 * D], BF16, kind="ExternalInput")
    wo = nc.dram_tensor("wo", [D, H * E], BF16, kind="ExternalInput")
    out = nc.dram_tensor("out", [S, E], BF16, kind="ExternalOutput")

    with tile.TileContext(nc) as tc:
        with (
            tc.tile_pool(name="const", bufs=1) as const,
            tc.tile_pool(name="work", bufs=2) as work,
            tc.tile_pool(name="psum", bufs=2, space="PSUM") as psum,
        ):
            wq_sb = const.tile([128, H * D], BF16)
            # split so the first q-proj matmul (needs cols 0:128 only) can
            # start as soon as the small slice lands
            nc.sync.dma_start(out=wq_sb[:, 0:128], in_=wq[:, 0:128])
            wkv_sb = const.tile([128, H * 2 * D], BF16)
            ones_col = const.tile([128, 1], BF16)
            nc.vector.memset(ones_col[:], 1.0)
            warm = const.tile([128, 1], F32)
            nc.vector.memset(warm[:], 0.0)
            nc.scalar.activation(warm[:], warm[:], AF.Exp)
            pqT = const.tile([128, H * S], BF16)      # all heads
            kv_all = const.tile([128, H * 129], BF16)  # all heads [kv|ksum]

            ctxT_tiles = {}
            ctx_tiles = {}

            def emit_nd_chunk(sc, h):
                ctx_sc = ctx_tiles[sc]
                # pass-B chunks alternate between the two 1-bank rings for
                # effective depth 4; pass-A chunks stay off "q2" (qp owns it)
                ndp = psum.tile([128, 129], F32,
                                tag="g" if (sc < 2 or h % 2 == 0) else "q2",
                                bufs=2)
                nc.tensor.matmul(
                    ndp[:],
                    pqT[:, h * S + sc * 128:h * S + (sc + 1) * 128],
                    kv_all[:, h * 129:(h + 1) * 129],
                    start=True, stop=True)
                inv = work.tile([128, 1], F32, tag="inv", bufs=4)
                nc.vector.reciprocal(inv[:], ndp[:, 128:129])
                dst = ctx_sc[:, h, :]
                if (sc + h) % 2 == 0:
                    nc.scalar.activation(dst, ndp[:, 0:128], AF.Copy,
                                         scale=inv[:, 0:1])
                else:
                    nc.vector.tensor_scalar(dst, ndp[:, 0:128],
                                            inv[:, 0:1], None, ALU.mult)

            xh2_tiles = {}

            xh2_pending = []

            def prefetch_q2(h, quarter):
                xh2 = work.tile([128, 512], BF16, tag="xh2", bufs=3)
                nc.sync.dma_start(
                    out=xh2[:],
                    in_=xT[h * 128:(h + 1) * 128,
                           quarter * 512:(quarter + 1) * 512])
                xh2_pending.append(xh2)

            def emit_q2_half(h, quarter):
                xh2 = xh2_pending.pop(0)
                q2p = psum.tile([128, 512], F32, tag="q2", bufs=2)
                nc.tensor.matmul(
                    q2p[:], wq_sb[:, h * 128:(h + 1) * 128],
                    xh2[:], start=True, stop=True)
                base = h * S + quarter * 512
                # Both PSUM reads on Act (releases q2p fast); DVE ops are
                # bf16-SBUF (4x/2x eligible) and off the q2p critical path.
                e2 = work.tile([128, 512], BF16, tag="e", bufs=2)
                nc.scalar.activation(e2[:], q2p[:], AF.Exp)
                r2 = work.tile([128, 512], BF16, tag="t", bufs=2)
                nc.scalar.activation(r2[:], q2p[:], AF.Relu)
                nc.vector.scalar_tensor_tensor(
                    pqT[:, base:base + 512], r2[:], 1.0, e2[:],
                    ALU.add, ALU.min)

            def emit_transpose(sc):
                ctx_sc = ctx_tiles.pop(sc)
                ctxT_sc = work.tile([128, H, 128], BF16, tag="ctxT", bufs=2)
                nc.sync.dma_start(
                    out=ctxT_sc[:],
                    in_=ctx_sc[:].rearrange("p h j -> p (h j)"),
                    transpose=True)
                ctxT_tiles[sc] = ctxT_sc

            def emit_nd(sc):
                ctx_sc = work.tile([128, H, 128], BF16, tag="ctx", bufs=2)
                ctx_tiles[sc] = ctx_sc
                for h in range(H):
                    emit_nd_chunk(sc, h)
                emit_transpose(sc)

            def emit_accum_eop(ctxT_sc, sc, eop, mid_hook=None, nd_hook=None):
                for eop in [eop]:
                    pa = psum.tile([128, 1024], F32, tag="pj", bufs=2)
                    for h in range(H):
                        if h in (5, 11) and mid_hook is not None:
                            mid_hook()
                        if nd_hook is not None:
                            nd_hook(h)
                        for half in range(2):
                            eoq = eop * 2 + half
                            nc.tensor.matmul(
                                pa[:, half * 512:(half + 1) * 512],
                                ctxT_sc[:, h, :], woq_tiles[eoq][:, h, :],
                                start=(h == 0), stop=(h == H - 1))
                    out_t = work.tile([128, 1024], BF16, tag="outsb",
                                       bufs=2)
                    if eop % 2 == 0:
                        nc.scalar.activation(out_t[:], pa[:], AF.Copy)
                    else:
                        nc.vector.tensor_copy(out_t[:], pa[:])
                    nc.sync.dma_start(
                        out=out[sc * 128:(sc + 1) * 128,
                                eop * 1024:(eop + 1) * 1024],
                        in_=out_t[:])

            wo_v = wo[:].rearrange("p (h q x) -> p h q x", q=4, x=512)
            woq_tiles = []

            # -------- Pass A (software-pipelined over heads) --------------
            # stage 1 (head h): q/k projections + phi; stage 2 (head h-1):
            # Gram/ksum/kv + early nd chunks.  The one-head lag hides the
            # phi latency so the PE never waits on the DVE/Act/Pool chains.
            pa_state = {}

            def emit_qk(h):
                xhT = work.tile([128, S], BF16, tag="xhT", bufs=2)
                if h == 0:
                    nc.sync.dma_start(out=xhT[:, 0:512], in_=xT[0:128, 0:512])
                    nc.sync.dma_start(out=wkv_sb[:, 0:256], in_=wkv[:, 0:256])
                    nc.sync.dma_start(out=xhT[:, 512:S],
                                      in_=xT[0:128, 512:S])
                else:
                    nc.sync.dma_start(out=xhT[:],
                                      in_=xT[h * 128:(h + 1) * 128, :])
                xn_sb = work.tile([128, NCH, 128], BF16, tag="xn", bufs=3)
                nc.sync.dma_start(
                    out=xn_sb[:],
                    in_=xn[h].rearrange("p (c j) -> p c j", j=128))
                if h == 0:
                    nc.sync.dma_start(out=wq_sb[:, 128:H * D],
                                      in_=wq[:, 128:H * D])
                    nc.sync.dma_start(out=wkv_sb[:, 256:],
                                      in_=wkv[:, 256:])

                # q quarter 0 + phi -> pqT[h, 0:512]; quarters 1-3 deferred
                # into pass B (emit_q2_half)
                qp = psum.tile([128, 512], F32, tag="q2", bufs=2)
                nc.tensor.matmul(
                    qp[:], wq_sb[:, h * 128:(h + 1) * 128],
                    xhT[:, 0:512], start=True, stop=True)
                eq = work.tile([128, 512], BF16, tag="e", bufs=2)
                nc.scalar.activation(eq[:], qp[:], AF.Exp)
                tq = work.tile([128, 512], BF16, tag="t", bufs=2)
                nc.vector.tensor_scalar(tq[:], qp[:], 0.0, 1.0,
                                        ALU.max, ALU.add)
                nc.vector.tensor_tensor(pqT[:, h * S:h * S + 512],
                                        eq[:], tq[:], ALU.min)

                # k (natural layout) + phi -> pk; mins go to the (otherwise
                # idle) GPSIMD engine to unload the DVE
                pk = work.tile([128, S], BF16, tag="pk", bufs=3)
                for j in range(2):
                    kp = psum.tile([128, 1024], F32, tag="pj", bufs=2)
                    for c in range(8):
                        sc = j * 8 + c
                        nc.tensor.matmul(
                            kp[:, c * 128:(c + 1) * 128],
                            xhT[:, sc * 128:(sc + 1) * 128],
                            wkv_sb[:, h * 256:h * 256 + 128],
                            start=True, stop=True)
                    e = work.tile([128, 1024], BF16, tag="e", bufs=2)
                    nc.scalar.activation(e[:], kp[:], AF.Exp)
                    t = work.tile([128, 1024], BF16, tag="t", bufs=2)
                    nc.vector.tensor_scalar(t[:], kp[:], 0.0, 1.0,
                                            ALU.max, ALU.add)
                    nc.gpsimd.tensor_tensor(
                        pk[:, j * 1024:(j + 1) * 1024], e[:], t[:], ALU.min)
                pa_state[h] = (pk, xn_sb)

            def emit_gram(h):
                pk, xn_sb = pa_state.pop(h)
                # one 1-bank psum packs G | kv | ksum: [0:128]=G, [128:256]=kv,
                # [256:257]=ksum, so kv_all[h] copies out in one op
                gkv = psum.tile([128, 257], F32, tag="g", bufs=2)
                for c in range(NCH):
                    nc.tensor.matmul(gkv[:, 0:128], xn_sb[:, c, :],
                                     pk[:, c * 128:(c + 1) * 128],
                                     start=(c == 0), stop=(c == NCH - 1))
                    nc.tensor.matmul(gkv[:, 256:257],
                                     pk[:, c * 128:(c + 1) * 128],
                                     ones_col[:],
                                     start=(c == 0), stop=(c == NCH - 1))
                g_sb = work.tile([128, 128], BF16, tag="gsb")
                nc.scalar.activation(g_sb[:], gkv[:, 0:128], AF.Copy)
                nc.tensor.matmul(gkv[:, 128:256], g_sb[:],
                                 wkv_sb[:, h * 256 + 128:h * 256 + 256],
                                 start=True, stop=True)
                nc.scalar.activation(
                    kv_all[:, h * 129:(h + 1) * 129], gkv[:, 128:257],
                    AF.Copy)
                if h == 0:
                    for esc in range(2):
                        ctxe = work.tile([128, H, 128], BF16,
                                         tag="ctx", bufs=2)
                        ctx_tiles[esc] = ctxe
                for esc in range(2):
                    emit_nd_chunk(esc, h)

            for eoq in range(4):
                woq = work.tile([128, H, 512], BF16, tag=f"woq{eoq}", bufs=1)
                woq_tiles.append(woq)

            def load_woq_chunk(eoq, hb):
                nc.sync.dma_start(
                    out=woq_tiles[eoq][:, hb * 4:(hb + 1) * 4, :],
                    in_=wo_v[:, hb * 4:(hb + 1) * 4, eoq, :])

            for h in range(H):
                emit_qk(h)
                if h > 1:
                    emit_gram(h - 2)
                if h >= 8:
                    # spread eoq0/1 weight loads over late pass A in 4-head
                    # chunks so no single transfer blocks the x loads
                    load_woq_chunk((h - 8) // 4, h % 4)
            emit_gram(H - 2)
            emit_gram(H - 1)

            emit_transpose(0)
            emit_transpose(1)
            # eoq2/3 weight chunks stream in at pass-B start; first needed
            # by eop1(0) one eop-length in
            for hb in range(4):
                load_woq_chunk(2, hb)
            for hb in range(4):
                load_woq_chunk(3, hb)
            q2q = [(h, q) for q in (1, 2, 3) for h in range(H)]
            q2q.reverse()  # pop() from the front

            q2pf = list(q2q)

            def drain_q2(n):
                for _ in range(n):
                    while q2pf and len(xh2_pending) < 3:
                        hq = q2pf.pop()
                        prefetch_q2(*hq)
                    if q2q:
                        h, half = q2q.pop()
                        emit_q2_half(h, half)

            for sc in range(2, NCH + 2):
                drain_q2(1)
                # nd chunks for sc interleave into the Wo stream of sc-2 so
                # the PE never head-of-line blocks on the nd scale chain
                if sc < NCH:
                    ctx_sc = work.tile([128, H, 128], BF16, tag="ctx", bufs=2)
                    ctx_tiles[sc] = ctx_sc
                    nd_hook = lambda h, _sc=sc: emit_nd_chunk(_sc, h)
                else:
                    nd_hook = None
                ctxT_sc = ctxT_tiles.pop(sc - 2)
                emit_accum_eop(ctxT_sc, sc - 2, 0,
                               mid_hook=lambda: drain_q2(1),
                               nd_hook=nd_hook)
                if sc < NCH:
                    emit_transpose(sc)
                drain_q2(1)
                emit_accum_eop(ctxT_sc, sc - 2, 1,
                               mid_hook=lambda: drain_q2(1))
                drain_q2(2)

    nc.compile()
    return nc


def get_module():
    if "nc" not in _CACHED:
        _CACHED["nc"] = build_module()
    return _CACHED["nc"]


def _bf16(a):
    return np.ascontiguousarray(a).astype(ml_dtypes.bfloat16)


def prepare_in_maps(inputs, Wq, Wk, Wv, Wo, bo):
    """Host-side shard + layout prep. Returns per-core input maps."""
    wq_p = _bf16(np.transpose(np.asarray(Wq), (1, 0, 2)).reshape(D, H * D))
    wkv = np.concatenate([np.asarray(Wk), np.asarray(Wv)], axis=2)  # (H,d,2d)
    wkv_p = _bf16(np.transpose(wkv, (1, 0, 2)).reshape(D, H * 2 * D))
    wo_p = _bf16(np.transpose(np.asarray(Wo).reshape(H, D, E),
                              (1, 0, 2)).reshape(D, H * E))
    in_maps = []
    for b in range(N_CORES):
        xb = np.asarray(inputs[b])
        # xn packed per head: xn[h][p, c*128+j] = x[c*128+p, h*128+j]
        xn_p = _bf16(np.transpose(xb.reshape(NCH, 128, H, D),
                                  (2, 1, 0, 3)).reshape(H, 128, NCH * D))
        in_maps.append({"xT": _bf16(xb.T), "xn": xn_p,
                        "wq": wq_p, "wkv": wkv_p, "wo": wo_p})
    return in_maps


def kernel(inputs, Wq, Wk, Wv, Wo, bo):
    B = inputs.shape[0]
    assert B == N_CORES and inputs.shape[1:] == (S, E)
    nc = get_module()
    in_maps = prepare_in_maps(inputs, Wq, Wk, Wv, Wo, bo)
    res = run_bass_kernel_spmd(nc, in_maps, list(range(N_CORES)))
    outs = np.stack([np.asarray(res.results[b]["out"], dtype=np.float32)
                     for b in range(N_CORES)], axis=0)
    return (outs + np.asarray(bo, dtype=np.float32)[None, None, :]).astype(
        np.float32)



# revision 18
# speedup vs baseline: 1.0960x; 1.0513x over previous
"""Trainium2 Bass kernel for nn_MultiHeadLinearAttention.

Full-input contract: kernel(**inputs) takes the unsharded numpy inputs and
returns the full output. Internally: data-parallel over batch across the 8
NeuronCores (B == 8, one batch element per core), no collectives.

Per-core math (S=2048, E=2048, H=16, d=128), bf16 matmuls + fp32 PSUM:
  Pass A (per head):
    qT  = Wq[h]-stationary route            -> pqT [d, S] (kept for all heads)
    k   = xT-chunk-stationary route         -> pk  [S, d] (natural, transient)
    phi(x) = elu(x)+1 = min(exp(x), 1 + relu(x))   (exact identity)
    Gram trick: G[d', d] = sum_s xh[s,d'] pk[s,d]  (16 accum. matmuls)
                kv = G^T @ Wv[h]  (one matmul; never materializes v)
    ksum[d]  = sum_s pk[s,d]  (N=1 matmuls into kv psum col 128)
  Pass B (per s-chunk sc):
    for each head: nd[s,129] = pqT-chunk^T @ [kv|ksum]; ctx_h = num * inv
    one batched xbar-DMA transpose of ctx_sc[s, (h d)] -> ctxT_sc[d, (h s)]
    out[sc, :] = sum_h ctxT_sc[:, h, :]^T @ Wo[h-rows, :]  (Wo streamed in
    eo-quarters)
Host does: x transpose + bf16 casts + weight packing + bias add + gather.
"""

import numpy as np
import ml_dtypes

import concourse.bass as bass
import concourse.mybir as mybir
import concourse.tile as tile
from concourse import bacc
from concourse.bass_utils import run_bass_kernel_spmd

S = 2048
E = 2048
H = 16
D = 128
N_CORES = 8
NCH = S // 128  # 16 s-chunks

F32 = mybir.dt.float32
BF16 = mybir.dt.bfloat16
AF = mybir.ActivationFunctionType
ALU = mybir.AluOpType

_CACHED = {}


def _phi(nc, work, psum_tile, dst):
    """phi(x) = min(exp(x), 1 + relu(x)) from a [128,1024] PSUM tile into
    bf16 SBUF dst (exact identity for elu(x)+1)."""
    e = work.tile([128, 1024], BF16, tag="e", bufs=2)
    nc.scalar.activation(e[:], psum_tile[:], AF.Exp)
    t = work.tile([128, 1024], BF16, tag="t", bufs=2)
    nc.vector.tensor_scalar(t[:], psum_tile[:], 0.0, 1.0, ALU.max, ALU.add)
    nc.vector.tensor_tensor(dst, e[:], t[:], ALU.min)


def build_module():
    nc = bacc.Bacc("TRN2", target_bir_lowering=False, debug=False,
                   num_devices=N_CORES)

    xT = nc.dram_tensor("xT", [E, S], BF16, kind="ExternalInput")
    xn = nc.dram_tensor("xn", [H, 128, NCH * 128], BF16,
                        kind="ExternalInput")
    wq = nc.dram_tensor("wq", [D, H * D], BF16, kind="ExternalInput")
    wkv = nc.dram_tensor("wkv", [D, H * 2 * D], BF16, kind="ExternalInput")
    wo = nc.dram_tensor("wo", [D, H * E], BF16, kind="ExternalInput")
    out = nc.dram_tensor("out", [S, E], BF16, kind="ExternalOutput")

    with tile.TileContext(nc) as tc:
        with (
            tc.tile_pool(name="const", bufs=1) as const,
            tc.tile_pool(name="work", bufs=2) as work,
            tc.tile_pool(name="psum", bufs=2, space="PSUM") as psum,
        ):
            wq_sb = const.tile([128, H * D], BF16)
            # split so the first q-proj matmul (needs cols 0:128 only) can
            # start as soon as the small slice lands
            nc.sync.dma_start(out=wq_sb[:, 0:128], in_=wq[:, 0:128])
            wkv_sb = const.tile([128, H * 2 * D], BF16)
            ones_col = const.tile([128, 1], BF16)
            nc.vector.memset(ones_col[:], 1.0)
            warm = const.tile([128, 1], F32)
            nc.vector.memset(warm[:], 0.0)
            nc.scalar.activation(warm[:], warm[:], AF.Exp)
            pqT = const.tile([128, H * S], BF16)      # all heads
            kv_all = const.tile([128, H * 129], BF16)  # all heads [kv|ksum]

            ctxT_tiles = {}
            ctx_tiles = {}

            def emit_nd_chunk(sc, h):
                ctx_sc = ctx_tiles[sc]
                # pass-B chunks alternate between the two 1-bank rings for
                # effective depth 4; pass-A chunks stay off "q2" (qp owns it)
                ndp = psum.tile([128, 129], F32,
                                tag="g" if (sc < 2 or h % 2 == 0) else "q2",
                                bufs=2)
                nc.tensor.matmul(
                    ndp[:],
                    pqT[:, h * S + sc * 128:h * S + (sc + 1) * 128],
                    kv_all[:, h * 129:(h + 1) * 129],
                    start=True, stop=True)
                inv = work.tile([128, 1], F32, tag="inv", bufs=4)
                nc.vector.reciprocal(inv[:], ndp[:, 128:129])
                dst = ctx_sc[:, h, :]
                if (sc + h) % 2 == 0:
                    nc.scalar.activation(dst, ndp[:, 0:128], AF.Copy,
                                         scale=inv[:, 0:1])
                else:
                    nc.vector.tensor_scalar(dst, ndp[:, 0:128],
                                            inv[:, 0:1], None, ALU.mult)

            xh2_tiles = {}

            xh2_pending = []

            def prefetch_q2(h, quarter):
                xh2 = work.tile([128, 512], BF16, tag="xh2", bufs=3)
                nc.sync.dma_start(
                    out=xh2[:],
                    in_=xT[h * 128:(h + 1) * 128,
                           quarter * 512:(quarter + 1) * 512])
                xh2_pending.append(xh2)

            def emit_q2_half(h, quarter):
                xh2 = xh2_pending.pop(0)
                q2p = psum.tile([128, 512], F32, tag="q2", bufs=2)
                nc.tensor.matmul(
                    q2p[:], wq_sb[:, h * 128:(h + 1) * 128],
                    xh2[:], start=True, stop=True)
                base = h * S + quarter * 512
                # Both PSUM reads on Act (releases q2p fast); DVE ops are
                # bf16-SBUF (4x/2x eligible) and off the q2p critical path.
                e2 = work.tile([128, 512], BF16, tag="et", bufs=8)
                nc.scalar.activation(e2[:], q2p[:], AF.Exp)
                r2 = work.tile([128, 512], BF16, tag="et", bufs=8)
                nc.scalar.activation(r2[:], q2p[:], AF.Relu)
                nc.vector.scalar_tensor_tensor(
                    pqT[:, base:base + 512], r2[:], 1.0, e2[:],
                    ALU.add, ALU.min)

            def emit_transpose(sc):
                ctx_sc = ctx_tiles.pop(sc)
                ctxT_sc = work.tile([128, H, 128], BF16, tag="ctxT", bufs=2)
                nc.sync.dma_start(
                    out=ctxT_sc[:],
                    in_=ctx_sc[:].rearrange("p h j -> p (h j)"),
                    transpose=True)
                ctxT_tiles[sc] = ctxT_sc

            def emit_nd(sc):
                ctx_sc = work.tile([128, H, 128], BF16, tag="ctx", bufs=2)
                ctx_tiles[sc] = ctx_sc
                for h in range(H):
                    emit_nd_chunk(sc, h)
                emit_transpose(sc)

            def emit_accum_eop(ctxT_sc, sc, eop, mid_hook=None, nd_hook=None):
                for eop in [eop]:
                    pa0 = psum.tile([128, 512], F32, tag="pj", bufs=4)
                    pa1 = psum.tile([128, 512], F32, tag="pj", bufs=4)
                    pas = (pa0, pa1)
                    for h in range(H):
                        if h in (5, 11) and mid_hook is not None:
                            mid_hook()
                        if nd_hook is not None:
                            nd_hook(h)
                        for half in range(2):
                            eoq = eop * 2 + half
                            nc.tensor.matmul(
                                pas[half][:],
                                ctxT_sc[:, h, :], woq_tiles[eoq][:, h, :],
                                start=(h == 0), stop=(h == H - 1))
                    out_t = work.tile([128, 1024], BF16, tag="outsb",
                                       bufs=2)
                    for half in range(2):
                        dst = out_t[:, half * 512:(half + 1) * 512]
                        if (eop + half) % 2 == 0:
                            nc.scalar.activation(dst, pas[half][:], AF.Copy)
                        else:
                            nc.vector.tensor_copy(dst, pas[half][:])
                    nc.sync.dma_start(
                        out=out[sc * 128:(sc + 1) * 128,
                                eop * 1024:(eop + 1) * 1024],
                        in_=out_t[:])

            wo_v = wo[:].rearrange("p (h q x) -> p h q x", q=4, x=512)
            woq_tiles = []

            # -------- Pass A (software-pipelined over heads) --------------
            # stage 1 (head h): q/k projections + phi; stage 2 (head h-1):
            # Gram/ksum/kv + early nd chunks.  The one-head lag hides the
            # phi latency so the PE never waits on the DVE/Act/Pool chains.
            pa_state = {}

            def emit_qk(h):
                xhT = work.tile([128, S], BF16, tag="xhT", bufs=2)
                if h == 0:
                    nc.sync.dma_start(out=xhT[:, 0:512], in_=xT[0:128, 0:512])
                    nc.sync.dma_start(out=wkv_sb[:, 0:256], in_=wkv[:, 0:256])
                    nc.sync.dma_start(out=xhT[:, 512:S],
                                      in_=xT[0:128, 512:S])
                else:
                    nc.sync.dma_start(out=xhT[:],
                                      in_=xT[h * 128:(h + 1) * 128, :])
                xn_sb = work.tile([128, NCH, 128], BF16, tag="xn", bufs=3)
                nc.sync.dma_start(
                    out=xn_sb[:],
                    in_=xn[h].rearrange("p (c j) -> p c j", j=128))
                if h == 0:
                    nc.sync.dma_start(out=wq_sb[:, 128:H * D],
                                      in_=wq[:, 128:H * D])
                    nc.sync.dma_start(out=wkv_sb[:, 256:],
                                      in_=wkv[:, 256:])

                # q quarter 0 + phi -> pqT[h, 0:512]; quarters 1-3 deferred
                # into pass B (emit_q2_half)
                qp = psum.tile([128, 512], F32, tag="q2", bufs=2)
                nc.tensor.matmul(
                    qp[:], wq_sb[:, h * 128:(h + 1) * 128],
                    xhT[:, 0:512], start=True, stop=True)
                eq = work.tile([128, 512], BF16, tag="et", bufs=8)
                nc.scalar.activation(eq[:], qp[:], AF.Exp)
                tq = work.tile([128, 512], BF16, tag="et", bufs=8)
                nc.vector.tensor_scalar(tq[:], qp[:], 0.0, 1.0,
                                        ALU.max, ALU.add)
                nc.vector.tensor_tensor(pqT[:, h * S:h * S + 512],
                                        eq[:], tq[:], ALU.min)

                # k (natural layout) + phi -> pk; mins go to the (otherwise
                # idle) GPSIMD engine to unload the DVE
                pk = work.tile([128, S], BF16, tag="pk", bufs=3)
                for j in range(4):
                    kp = psum.tile([128, 512], F32, tag="pj", bufs=4)
                    for c in range(4):
                        sc = j * 4 + c
                        nc.tensor.matmul(
                            kp[:, c * 128:(c + 1) * 128],
                            xhT[:, sc * 128:(sc + 1) * 128],
                            wkv_sb[:, h * 256:h * 256 + 128],
                            start=True, stop=True)
                    e = work.tile([128, 512], BF16, tag="et", bufs=8)
                    nc.scalar.activation(e[:], kp[:], AF.Exp)
                    t = work.tile([128, 512], BF16, tag="et", bufs=8)
                    nc.vector.tensor_scalar(t[:], kp[:], 0.0, 1.0,
                                            ALU.max, ALU.add)
                    nc.gpsimd.tensor_tensor(
                        pk[:, j * 512:(j + 1) * 512], e[:], t[:], ALU.min)
                pa_state[h] = (pk, xn_sb)

            def emit_gram(h):
                pk, xn_sb = pa_state.pop(h)
                # one 1-bank psum packs G | kv | ksum: [0:128]=G, [128:256]=kv,
                # [256:257]=ksum, so kv_all[h] copies out in one op
                gkv = psum.tile([128, 257], F32, tag="g", bufs=2)
                for c in range(NCH):
                    nc.tensor.matmul(gkv[:, 0:128], xn_sb[:, c, :],
                                     pk[:, c * 128:(c + 1) * 128],
                                     start=(c == 0), stop=(c == NCH - 1))
                    nc.tensor.matmul(gkv[:, 256:257],
                                     pk[:, c * 128:(c + 1) * 128],
                                     ones_col[:],
                                     start=(c == 0), stop=(c == NCH - 1))
                g_sb = work.tile([128, 128], BF16, tag="gsb")
                nc.scalar.activation(g_sb[:], gkv[:, 0:128], AF.Copy)
                nc.tensor.matmul(gkv[:, 128:256], g_sb[:],
                                 wkv_sb[:, h * 256 + 128:h * 256 + 256],
                                 start=True, stop=True)
                nc.scalar.activation(
                    kv_all[:, h * 129:(h + 1) * 129], gkv[:, 128:257],
                    AF.Copy)
                if h == 0:
                    for esc in range(2):
                        ctxe = work.tile([128, H, 128], BF16,
                                         tag="ctx", bufs=2)
                        ctx_tiles[esc] = ctxe
                for esc in range(2):
                    emit_nd_chunk(esc, h)

            for eoq in range(4):
                woq = work.tile([128, H, 512], BF16, tag=f"woq{eoq}", bufs=1)
                woq_tiles.append(woq)

            def load_woq_chunk(eoq, hb):
                nc.sync.dma_start(
                    out=woq_tiles[eoq][:, hb * 4:(hb + 1) * 4, :],
                    in_=wo_v[:, hb * 4:(hb + 1) * 4, eoq, :])

            for h in range(H):
                emit_qk(h)
                if h > 1:
                    emit_gram(h - 2)
                if h >= 8:
                    # spread eoq0/1 weight loads over late pass A in 4-head
                    # chunks so no single transfer blocks the x loads
                    load_woq_chunk((h - 8) // 4, h % 4)
            emit_gram(H - 2)
            emit_gram(H - 1)

            emit_transpose(0)
            emit_transpose(1)
            # eoq2/3 weight chunks stream in at pass-B start; first needed
            # by eop1(0) one eop-length in
            for hb in range(4):
                load_woq_chunk(2, hb)
            for hb in range(4):
                load_woq_chunk(3, hb)
            q2q = [(h, q) for q in (1, 2, 3) for h in range(H)]
            q2q.reverse()  # pop() from the front

            q2pf = list(q2q)

            def drain_q2(n):
                for _ in range(n):
                    while q2pf and len(xh2_pending) < 3:
                        hq = q2pf.pop()
                        prefetch_q2(*hq)
                    if q2q:
                        h, half = q2q.pop()
                        emit_q2_half(h, half)

            for sc in range(2, NCH + 2):
                drain_q2(1)
                # nd chunks for sc interleave into the Wo stream of sc-2 so
                # the PE never head-of-line blocks on the nd scale chain
                if sc < NCH:
                    ctx_sc = work.tile([128, H, 128], BF16, tag="ctx", bufs=2)
                    ctx_tiles[sc] = ctx_sc
                    nd_hook = lambda h, _sc=sc: emit_nd_chunk(_sc, h)
                else:
                    nd_hook = None
                ctxT_sc = ctxT_tiles.pop(sc - 2)
                emit_accum_eop(ctxT_sc, sc - 2, 0,
                               mid_hook=lambda: drain_q2(1),
                               nd_hook=nd_hook)
                if sc < NCH:
                    emit_transpose(sc)
                drain_q2(1)
                emit_accum_eop(ctxT_sc, sc - 2, 1,
                               mid_hook=lambda: drain_q2(1))
                drain_q2(2)

    nc.compile()
    return nc


def get_module():
    if "nc" not in _CACHED:
        _CACHED["nc"] = build_module()
    return _CACHED["nc"]


def _bf16(a):
    return np.ascontiguousarray(a).astype(ml_dtypes.bfloat16)


def prepare_in_maps(inputs, Wq, Wk, Wv, Wo, bo):
    """Host-side shard + layout prep. Returns per-core input maps."""
    wq_p = _bf16(np.transpose(np.asarray(Wq), (1, 0, 2)).reshape(D, H * D))
    wkv = np.concatenate([np.asarray(Wk), np.asarray(Wv)], axis=2)  # (H,d,2d)
    wkv_p = _bf16(np.transpose(wkv, (1, 0, 2)).reshape(D, H * 2 * D))
    wo_p = _bf16(np.transpose(np.asarray(Wo).reshape(H, D, E),
                              (1, 0, 2)).reshape(D, H * E))
    in_maps = []
    for b in range(N_CORES):
        xb = np.asarray(inputs[b])
        # xn packed per head: xn[h][p, c*128+j] = x[c*128+p, h*128+j]
        xn_p = _bf16(np.transpose(xb.reshape(NCH, 128, H, D),
                                  (2, 1, 0, 3)).reshape(H, 128, NCH * D))
        in_maps.append({"xT": _bf16(xb.T), "xn": xn_p,
                        "wq": wq_p, "wkv": wkv_p, "wo": wo_p})
    return in_maps


def kernel(inputs, Wq, Wk, Wv, Wo, bo):
    B = inputs.shape[0]
    assert B == N_CORES and inputs.shape[1:] == (S, E)
    nc = get_module()
    in_maps = prepare_in_maps(inputs, Wq, Wk, Wv, Wo, bo)
    res = run_bass_kernel_spmd(nc, in_maps, list(range(N_CORES)))
    outs = np.stack([np.asarray(res.results[b]["out"], dtype=np.float32)
                     for b in range(N_CORES)], axis=0)
    return (outs + np.asarray(bo, dtype=np.float32)[None, None, :]).astype(
        np.float32)

